# revision 4
# baseline (speedup 1.0000x reference)
"""Trainium2 Bass kernel for nn_EnergyDistributionCNN (3x3 conv -> unfold ->
softmax over patch -> weighted -> fold overlap-add), 8 NeuronCores.

Math (algebraically identical to the torch/jax reference):
    out = conv3x3(x, k)            cross-correlation, zero pad 1
    E   = exp(out)
    Z   = boxsum3x3(E padded with ONES)   (zero pads contribute exp(0)=1)
    U   = x / Z
    S   = boxsum3x3(U zero-padded)
    result = E * S

Sharding: row-block across 8 cores with a 3-row halo sliced on the host
(zero-filled at the global top/bottom edges), so no device-to-device
communication is needed. Global boundary semantics are handled uniformly by
a per-row mask: E = exp(mask*out) gives exp(0)=1 in out-of-grid rows, and
out-of-grid x rows are zero so U's zero-padding is automatic.

On-core layout: rows on partitions, columns on the free dim. All vertical
(partition-direction) stencil mixing runs on the TensorEngine via small
banded matrices (built on the host from `kernel`); horizontal mixing is 3
column-shifted matmuls accumulated in PSUM. exp on the ScalarEngine reads
the conv's PSUM directly (fused with the boundary mask via its per-partition
scale); 1/Z uses the DVE's fast reciprocal. Band row-mappings are chosen so
every compute op sees partition base 0; the final output rows sit at
partitions [2, R+2), which only the (unrestricted) output DMA reads.
"""

from contextlib import ExitStack

import numpy as np

import concourse.bacc as bacc
import concourse.mybir as mybir
import concourse.tile as tile
from concourse._compat import with_exitstack
from concourse.bass_utils import run_bass_kernel_spmd

F32 = mybir.dt.float32
# fp32r: PE runs the moving operand at full rate (1 col/cycle vs 2 for fp32)
# at ~12-13 effective mantissa bits -- measured end-to-end error ~5e-4 rel,
# far inside the gate. All matmul operands (X, E, U, bands) use it.
MDT = mybir.dt.float32r

H = 4096
W = 4096
N_CORES = 8
RC = H // N_CORES  # rows per core
HALO = 3
RT = 122  # output rows per tile (RT + 6 <= 128 partitions)
C = 512  # matmul column chunk = one fp32 PSUM bank


# ---------------------------------------------------------------- host side

def _make_bands(k: np.ndarray) -> np.ndarray:
    """bands[v][p, m] = k[p-m, v] (conv, v=0..2); bands[3] = BB ones with
    p-m in 0..2 (S matmul); bands[4] = BT ones with m-p in 0..2 (Z matmul)."""
    bands = np.zeros((5, 128, 128), np.float32)
    idx = np.arange(128)
    for d in range(3):
        p = idx[d:]
        m = idx[: 128 - d]
        for v in range(3):
            bands[v, p, m] = k[d, v]
        bands[3, p, m] = 1.0
        bands[4, m, p] = 1.0
    return bands


def _make_core_inputs(x: np.ndarray, bands: np.ndarray, core: int):
    r0 = core * RC
    lo, hi = r0 - HALO, r0 + RC + HALO
    xh = np.zeros((RC + 2 * HALO, W), np.float32)
    s_lo, s_hi = max(lo, 0), min(hi, H)
    xh[s_lo - lo : s_hi - lo, :] = x[s_lo:s_hi]
    gl = np.arange(lo, hi)
    mask = ((gl >= 0) & (gl < H)).astype(np.float32)[:, None]
    return {"xh": xh, "mask": mask, "bands": bands}


def _make_tiles():
    tiles = []
    o = 0
    while o < RC:
        R = min(RT, RC - o)
        tiles.append((o, R))
        o += R
    return tiles


# -------------------------------------------------------------- device side

@with_exitstack
def _energy_body(ctx: ExitStack, tc, out_d, xh_d, mask_d, bands_d):
    nc = tc.nc
    nch = W // C
    Exp = mybir.ActivationFunctionType.Exp

    consts = ctx.enter_context(tc.tile_pool(name="consts", bufs=1))
    bands = []
    for i in range(5):
        b = consts.tile([128, 128], MDT, name=f"band{i}")
        nc.gpsimd.dma_start(out=b, in_=bands_d[i])
        bands.append(b)
    BB, BT = bands[3], bands[4]

    xpool = ctx.enter_context(tc.tile_pool(name="xp", bufs=2))
    epool = ctx.enter_context(tc.tile_pool(name="ep", bufs=2))
    rzpool = ctx.enter_context(tc.tile_pool(name="rzp", bufs=2))
    upool = ctx.enter_context(tc.tile_pool(name="up", bufs=2))
    respool = ctx.enter_context(tc.tile_pool(name="resp", bufs=2))
    mpool = ctx.enter_context(tc.tile_pool(name="mp", bufs=2))
    ps_conv = ctx.enter_context(tc.tile_pool(name="psc", bufs=2, space="PSUM"))
    ps_z = ctx.enter_context(tc.tile_pool(name="psz", bufs=2, space="PSUM"))
    ps_s = ctx.enter_context(tc.tile_pool(name="pss", bufs=2, space="PSUM"))

    for o, R in _make_tiles():
        # X[p] <-> row r-3+p (r = this tile's first output row)
        X = xpool.tile([128, W + 2], MDT, tag="X")
        nc.vector.memset(X[: R + 6, 0:1].bitcast(F32), 0.0)
        nc.vector.memset(X[: R + 6, W + 1 : W + 2].bitcast(F32), 0.0)
        nc.gpsimd.dma_start(out=X[: R + 6, 1 : W + 1], in_=xh_d[o : o + R + 6, :])

        mk = mpool.tile([128, 1], F32, tag="mk")
        nc.sync.dma_start(out=mk[: R + 4], in_=mask_d[o + 1 : o + R + 5, :])

        # conv psum / E[m] <-> row r-2+m: E = exp(mask * conv)
        E = epool.tile([128, W + 2], MDT, tag="E")
        nc.vector.memset(E[: R + 4, 0:1].bitcast(F32), 1.0)
        nc.vector.memset(E[: R + 4, W + 1 : W + 2].bitcast(F32), 1.0)
        for c in range(nch):
            pc = ps_conv.tile([128, C], F32, tag="pc")
            for v in range(3):
                nc.tensor.matmul(
                    pc[: R + 4, :],
                    bands[v][: R + 6, : R + 4],
                    X[: R + 6, c * C + v : c * C + v + C],
                    start=(v == 0),
                    stop=(v == 2),
                )
            nc.scalar.activation(
                E[: R + 4, 1 + c * C : 1 + (c + 1) * C],
                pc[: R + 4, :],
                Exp,
                scale=mk[: R + 4],
            )

        # Z[m] <-> row r-3+m (X frame): Z[m] = sum_{q=m-2..m} E[q]
        Rz = rzpool.tile([128, W], F32, tag="Rz")
        for c in range(nch):
            pz = ps_z.tile([128, C], F32, tag="pz")
            for v in range(3):
                nc.tensor.matmul(
                    pz[: R + 4, :],
                    BT[: R + 4, : R + 4],
                    E[: R + 4, c * C + v : c * C + v + C],
                    start=(v == 0),
                    stop=(v == 2),
                )
            nc.vector.reciprocal_approx_fast(
                out=Rz[: R + 4, c * C : (c + 1) * C], in_=pz[: R + 4, :]
            )

        # U[m] = X[m] * Rz[m]  (X frame, valid m in [2, R+4))
        U = upool.tile([128, W + 2], MDT, tag="U")
        nc.vector.memset(U[: R + 4, 0:1].bitcast(F32), 0.0)
        nc.vector.memset(U[: R + 4, W + 1 : W + 2].bitcast(F32), 0.0)
        nc.vector.tensor_mul(
            out=U[: R + 4, 1 : W + 1],
            in0=X[: R + 4, 1 : W + 1],
            in1=Rz[: R + 4, :W],
        )

        # S[m] <-> row r-2+m (E frame): S[m] = sum_{q=m..m+2} U[q]
        res = respool.tile([128, W], F32, tag="res")
        for c in range(nch):
            ps = ps_s.tile([128, C], F32, tag="ps")
            for v in range(3):
                nc.tensor.matmul(
                    ps[: R + 2, :],
                    BB[: R + 4, : R + 2],
                    U[: R + 4, c * C + v : c * C + v + C],
                    start=(v == 0),
                    stop=(v == 2),
                )
            nc.vector.tensor_mul(
                out=res[: R + 2, c * C : (c + 1) * C],
                in0=E[: R + 2, 1 + c * C : 1 + (c + 1) * C],
                in1=ps[: R + 2, :],
            )
        # valid output rows sit at partitions [2, R+2) -- DMA reads them
        nc.sync.dma_start(out=out_d[o : o + R, :], in_=res[2 : R + 2, :W])


_CACHE: dict = {}


def _build():
    if "nc" in _CACHE:
        return _CACHE["nc"]
    nc = bacc.Bacc(
        "TRN2", target_bir_lowering=False, debug=False, num_devices=N_CORES
    )
    xh_d = nc.dram_tensor("xh", (RC + 2 * HALO, W), F32, kind="ExternalInput").ap()
    mask_d = nc.dram_tensor("mask", (RC + 2 * HALO, 1), F32, kind="ExternalInput").ap()
    bands_d = nc.dram_tensor("bands", (5, 128, 128), F32, kind="ExternalInput").ap()
    out_d = nc.dram_tensor("out", (RC, W), F32, kind="ExternalOutput").ap()
    with tile.TileContext(nc) as tc:
        _energy_body(tc, out_d, xh_d, mask_d, bands_d)
    nc.compile()
    _CACHE["nc"] = nc
    return nc


def kernel(shareable_energy: np.ndarray, kernel: np.ndarray, **_run_kw) -> np.ndarray:
    x = np.ascontiguousarray(np.asarray(shareable_energy, np.float32))
    k = np.asarray(kernel, np.float32)
    assert x.shape == (H, W), x.shape
    nc = _build()
    bands = _make_bands(k)
    in_maps = [_make_core_inputs(x, bands, core) for core in range(N_CORES)]
    r = run_bass_kernel_spmd(
        nc, in_maps, core_ids=list(range(N_CORES)), **_run_kw
    )
    out = np.concatenate([res["out"] for res in r.results], axis=0)
    if _run_kw:
        _CACHE["last_result"] = r
    return out
